# revision 22
# baseline (speedup 1.0000x reference)
"""Trainium2 Bass kernel for nn_DiscriminatorLatent (dense MLP discriminator).

Strategy (pure data parallel over 8 NeuronCores, per the sharding hint):
 - batch dim (8192) sharded 1024 rows/core; weights replicated.
 - All on-chip tensors are feature-major (transposed): last^T is kept in SBUF
   as 128x1024 chunks; layer matmuls compute z^T = (W^T).T @ last^T in bf16
   with fp32 PSUM accumulation.
 - Each layer's matmul is split: A-phase contracts feature chunks that were
   finalized at least one layer ago (partials drained to SBUF fp32), B-phase
   contracts the most recent layer's chunks and adds the partial back.  This
   keeps the PE busy on A-phase work of the *next* layer while the current
   layer's BatchNorm AllReduce is in flight.
 - BatchNorm batch stats: per-core per-feature sum / sum-of-squares computed
   on ACT (fused into the PSUM drain + a square pass, both fp32-exact), then
   one [128,8] AllReduce per layer across the 8 cores -> exact full-batch
   statistics.  The linear bias b cancels exactly inside training-mode BN,
   so it is not applied; gamma/beta are applied.
 - LeakyReLU fused with the BN affine on ACT (Lrelu(z*scale+shift)); noise
   multiply on DVE (noise stays fp32, pre-transposed host-side so every DMA
   is contiguous).
 - Final logits: Wc contributions accumulated incrementally (M=1 matmuls per
   finished feature chunk) into PSUM, summed in SBUF, sigmoid on ACT.
"""

import os
import sys

if "/opt/trn_rl_repo" not in sys.path:
    sys.path.insert(0, "/opt/trn_rl_repo")

import ml_dtypes
import numpy as np

import concourse.bass as bass
import concourse.tile as tile
from concourse import bacc, mybir
from concourse import bass_utils

F32 = mybir.dt.float32
F32R = mybir.dt.float32r
BF16 = mybir.dt.bfloat16
AF = mybir.ActivationFunctionType
ALU = mybir.AluOpType

N_CORES = 8
B = 8192
B_LOC = B // N_CORES  # 1024
LVS = 512
WIDTH = 512
DEPTH = 7
EPS = 1e-5
SLOPE = 0.01
NH = B_LOC // 512  # moving-operand halves per z tile (N max = 512 per bank)

MM_DT = BF16  # matmul dtype for weights/activations (PSUM stays fp32)
MM_NP = ml_dtypes.bfloat16

# module-level knobs for the test harness
TRACE = False
LAST_EXEC_NS = None
LAST_RESULTS = None

_BUILD_CACHE = {}


def _build(depth=DEPTH):
    """Build + compile the SPMD bass program for `depth` layers."""
    nc = bacc.Bacc("TRN2", target_bir_lowering=False, debug=False,
                   num_devices=N_CORES)

    n_chunks = 4 * (depth + 1)  # 128-feature chunks in final last^T

    # ---- DRAM I/O ----------------------------------------------------------
    xt_d = nc.dram_tensor("xt", [LVS, B_LOC], MM_DT, kind="ExternalInput").ap()
    wt_d = [
        nc.dram_tensor(f"wt{i}", [LVS + WIDTH * i, WIDTH], MM_DT,
                       kind="ExternalInput").ap()
        for i in range(depth)
    ]
    noiset_d = nc.dram_tensor("noiset", [depth, WIDTH, B_LOC], F32,
                              kind="ExternalInput").ap()
    wct_d = nc.dram_tensor("wct", [128, n_chunks], MM_DT,
                           kind="ExternalInput").ap()
    gammat_d = nc.dram_tensor("gammat", [128, 4 * depth], F32,
                              kind="ExternalInput").ap()
    betat_d = nc.dram_tensor("betat", [128, 4 * depth], F32,
                             kind="ExternalInput").ap()
    bct_d = nc.dram_tensor("bct", [1, 1], F32, kind="ExternalInput").ap()
    out_d = nc.dram_tensor("out", [1, B_LOC], F32, kind="ExternalOutput").ap()

    # ---- persistent SBUF ---------------------------------------------------
    lastT = [
        nc.alloc_sbuf_tensor(f"lastT{k}", [128, B_LOC], MM_DT).ap()
        for k in range(n_chunks)
    ]
    wct_sb = nc.alloc_sbuf_tensor("wct_sb", [128, n_chunks], MM_DT).ap()
    gammat_sb = nc.alloc_sbuf_tensor("gammat_sb", [128, 4 * depth], F32).ap()
    betat_sb = nc.alloc_sbuf_tensor("betat_sb", [128, 4 * depth], F32).ap()
    bct_sb = nc.alloc_sbuf_tensor("bct_sb", [1, 1], F32).ap()
    logits_acc = nc.alloc_sbuf_tensor("logits_acc", [1, B_LOC], F32).ap()
    out_sb = nc.alloc_sbuf_tensor("out_sb", [1, B_LOC], F32).ap()

    with tile.TileContext(nc) as tc:
        with (
            tc.tile_pool(name="wpool", bufs=44) as wpool,
            tc.tile_pool(name="npool", bufs=4) as npool,
            tc.tile_pool(name="ppool", bufs=12) as ppool,
            tc.tile_pool(name="spool", bufs=2) as spool,
            tc.tile_pool(name="stpool", bufs=4) as stpool,
            tc.tile_pool(name="fpool", bufs=16) as fpool,
            tc.tile_pool(name="zpool", bufs=3, space="PSUM") as zpool,
            tc.tile_pool(name="lpool", bufs=1, space="PSUM") as lpool,
            tc.tile_pool(name="dpool", bufs=4, space="DRAM") as dpool,
            tc.tile_pool(name="xdpool", bufs=12, space="DRAM") as xdpool,
        ):
            # ---- preload constants + x ----
            eps_t = nc.alloc_sbuf_tensor("const_eps", [128, 1], F32)
            nc.gpsimd.memset(eps_t.ap(), EPS)
            nc.const_aps.aps[(F32, EPS)] = eps_t.ap()
            dum = nc.alloc_sbuf_tensor("dum", [128, 1], F32)
            nc.scalar.activation(dum.ap()[:], eps_t.ap()[:], AF.Sqrt, bias=EPS)
            nc.scalar.activation(dum.ap()[:], eps_t.ap()[:], AF.Lrelu,
                                 bias=0.0, scale=1.0, alpha=SLOPE)
            for k in range(4):
                nc.sync.dma_start(lastT[k][:], xt_d[k * 128:(k + 1) * 128, :])
            nc.sync.dma_start(wct_sb[:], wct_d[:])
            nc.sync.dma_start(gammat_sb[:], gammat_d[:])
            nc.sync.dma_start(betat_sb[:], betat_d[:])
            nc.sync.dma_start(bct_sb[:], bct_d[:])

            def emit_logits(group):
                """Accumulate Wc contribution of chunks 4g..4g+3 into logits."""
                lp = lpool.tile([1, B_LOC], F32)
                for jj in range(4):
                    j = 4 * group + jj
                    for h in range(NH):
                        nc.tensor.matmul(
                            lp[:, h * 512:(h + 1) * 512],
                            wct_sb[:, j:j + 1],
                            lastT[j][:, h * 512:(h + 1) * 512],
                            start=(jj == 0),
                            stop=(jj == 3),
                        )
                if group == 0:
                    nc.vector.tensor_copy(logits_acc[:], lp[:])
                else:
                    nc.vector.tensor_add(logits_acc[:], logits_acc[:], lp[:])

            def load_wblocks(i, ks):
                """Load contiguous [128,512] row-blocks k of W_i^T."""
                tiles = {}
                for k in ks:
                    wt = wpool.tile([128, WIDTH], MM_DT)
                    nc.sync.dma_start(
                        wt[:], wt_d[i][k * 128:(k + 1) * 128, :])
                    tiles[k] = wt
                return tiles

            def mm_accum_multi(psum_ts, wtiles, ms, ks):
                """Interleave accumulation of several m-tiles per k-block so
                the PE consumes each freshly-DMA'd weight block 3x slower
                than a single m-chain would (avoids burst starvation)."""
                for idx, k in enumerate(ks):
                    for m in ms:
                        for h in range(NH):
                            nc.tensor.matmul(
                                psum_ts[m][:, h * 512:(h + 1) * 512],
                                wtiles[k][:, m * 128:(m + 1) * 128],
                                lastT[k][:, h * 512:(h + 1) * 512],
                                start=(idx == 0),
                                stop=(idx == len(ks) - 1),
                            )

            def mm_accum(psum_t, wtiles, m, ks):
                for idx, k in enumerate(ks):
                    for h in range(NH):
                        nc.tensor.matmul(
                            psum_t[:, h * 512:(h + 1) * 512],
                            wtiles[k][:, m * 128:(m + 1) * 128],
                            lastT[k][:, h * 512:(h + 1) * 512],
                            start=(idx == 0),
                            stop=(idx == len(ks) - 1),
                        )

            def emit_norm(i, scale4, shift4):
                """normalize + LeakyReLU + noise + logits contribution for
                layer i's output chunks (per-chunk, so downstream consumers
                unblock as early as possible)."""
                new0 = 4 * (i + 1)
                lp = lpool.tile([1, B_LOC], F32)
                for m in range(4):
                    ch = lastT[new0 + m]
                    nc.scalar.activation(
                        ch[:], ch[:], AF.Lrelu,
                        bias=shift4[:, m:m + 1],
                        scale=scale4[:, m:m + 1],
                        alpha=SLOPE,
                    )
                    ntile = npool.tile([128, B_LOC], F32)
                    nc.scalar.dma_start(
                        ntile[:],
                        noiset_d[i:i + 1, m * 128:(m + 1) * 128, :],
                    )
                    nc.vector.tensor_mul(ch[:], ch[:], ntile[:])
                    for h in range(NH):
                        nc.tensor.matmul(
                            lp[:, h * 512:(h + 1) * 512],
                            wct_sb[:, new0 + m:new0 + m + 1],
                            ch[:, h * 512:(h + 1) * 512],
                            start=(m == 0),
                            stop=(m == 3),
                        )
                nc.vector.tensor_add(logits_acc[:], logits_acc[:], lp[:])

            def emit_stats_finalize(i, gstats):
                mean4 = fpool.tile([128, 4], F32)
                ex24 = fpool.tile([128, 4], F32)
                msq4 = fpool.tile([128, 4], F32)
                var4 = fpool.tile([128, 4], F32)
                std4 = fpool.tile([128, 4], F32)
                rstd4 = fpool.tile([128, 4], F32)
                scale4 = fpool.tile([128, 4], F32)
                nms4 = fpool.tile([128, 4], F32)
                shift4 = fpool.tile([128, 4], F32)
                nc.vector.tensor_scalar_mul(mean4[:], gstats[:, 0:4], 1.0 / B)
                nc.vector.tensor_scalar_mul(ex24[:], gstats[:, 4:8], 1.0 / B)
                nc.vector.tensor_mul(msq4[:], mean4[:], mean4[:])
                nc.vector.tensor_sub(var4[:], ex24[:], msq4[:])
                nc.scalar.activation(std4[:], var4[:], AF.Sqrt, bias=EPS)
                nc.vector.reciprocal(rstd4[:], std4[:])
                nc.vector.tensor_mul(
                    scale4[:], rstd4[:], gammat_sb[:, 4 * i:4 * i + 4])
                nc.vector.scalar_tensor_tensor(
                    nms4[:], mean4[:], -1.0, scale4[:],
                    op0=ALU.mult, op1=ALU.mult)
                nc.vector.tensor_add(
                    shift4[:], nms4[:], betat_sb[:, 4 * i:4 * i + 4])
                return scale4, shift4

            xparts = {}
            xparts_dram = {}

            def emit_xparts():
                """x's logits + pre-contraction of the x chunks of all later
                layers.  Emitted after layer 0 ships its stats: fills the PE
                during the collectives init barrier + first AllReduce.
                Layers 2/3 keep their partials in SBUF; 4+ bounce via DRAM."""
                emit_logits(0)
                for j in range(3, depth):
                    wtiles_x = load_wblocks(j, range(4))
                    tgt = xparts if j < 4 else xparts_dram
                    tgt[j] = {}
                    for m in range(4):
                        xt_ps = zpool.tile([128, B_LOC], F32, tag="z")
                        mm_accum(xt_ps, wtiles_x, m, range(4))
                        xp = ppool.tile([128, B_LOC], F32, tag="pt")
                        nc.vector.tensor_copy(xp[:], xt_ps[:])
                        if j < 4:
                            tgt[j][m] = xp
                        else:
                            xd = xdpool.tile([128, B_LOC], F32)
                            nc.gpsimd.dma_start(xd[:], xp[:])
                            tgt[j][m] = xd

            # ---- layer pipeline ----
            # A-phase of layer i: chunks 0..4i-1 (ready >= one layer ago)
            # B-phase of layer i: chunks 4i..4i+3 (previous layer's output)
            pending = None  # (i, lstats) shipped to AllReduce, not yet normed
            for i in range(depth):
                if i == 1:
                    emit_xparts()
                xpart = xparts.get(i)
                xpart_d = xparts_dram.get(i)
                old_ks = list(range(4 if (xpart or xpart_d) else 0, 4 * i))
                new_ks = list(range(4 * i, 4 * (i + 1)))

                # A-phase (independent of the pending AllReduce).  The m=3
                # drain is deferred past the retire chain so the DVE queue
                # head is free the moment the AllReduce lands.
                deferred_drain = None
                if old_ks:
                    wtiles_a = load_wblocks(i, old_ks)
                    if xpart_d:
                        xpart = {}
                        for m in range(4):
                            pt = ppool.tile([128, B_LOC], F32, tag="pt")
                            nc.gpsimd.dma_start(pt[:], xpart_d[m][:])
                            xpart[m] = pt
                    ats = {}
                    for m in range(3):
                        at_t = zpool.tile([128, B_LOC], F32, tag="z")
                        ats[m] = at_t
                    mm_accum_multi(ats, wtiles_a, (0, 1, 2), old_ks)
                    new_partials = {}
                    for m in range(3):
                        if xpart:
                            pt = xpart[m]
                            nc.vector.tensor_add(pt[:], pt[:], ats[m][:])
                        else:
                            pt = ppool.tile([128, B_LOC], F32, tag="pt")
                            nc.vector.tensor_copy(pt[:], ats[m][:])
                        new_partials[m] = pt
                    at3 = zpool.tile([128, B_LOC], F32, tag="z")
                    mm_accum(at3, wtiles_a, 3, old_ks)
                    if xpart:
                        deferred_drain = ("add", at3, xpart[3])
                        new_partials[3] = xpart[3]
                    else:
                        pt3 = ppool.tile([128, B_LOC], F32, tag="pt")
                        deferred_drain = ("copy", at3, pt3)
                        new_partials[3] = pt3
                else:
                    new_partials = xpart

                # retire the pending AllReduce: finalize + normalize + logits
                if pending is not None:
                    pi, gstats = pending
                    scale4, shift4 = emit_stats_finalize(pi, gstats)
                    emit_norm(pi, scale4, shift4)
                    pending = None
                if deferred_drain is not None:
                    kind, at, pt = deferred_drain
                    if kind == "add":
                        nc.vector.tensor_add(pt[:], pt[:], at[:])
                    else:
                        nc.vector.tensor_copy(pt[:], at[:])

                # B-phase: newest chunks + partial add, then stats
                wtiles_b = load_wblocks(i, new_ks)
                lstats = stpool.tile([128, 8], F32)
                for m in range(4):
                    bt = zpool.tile([128, B_LOC], F32, tag="z")
                    mm_accum(bt, wtiles_b, m, new_ks)
                    ch = lastT[4 * (i + 1) + m]
                    if new_partials is not None:
                        nc.vector.tensor_tensor(
                            ch[:], bt[:], new_partials[m][:], op=ALU.add)
                    else:
                        nc.vector.tensor_copy(ch[:], bt[:])
                    nc.vector.tensor_reduce(
                        lstats[:, m:m + 1], ch[:],
                        axis=mybir.AxisListType.X, op=ALU.add)
                    sq = spool.tile([128, B_LOC], BF16)
                    nc.gpsimd.tensor_mul(sq[:], ch[:], ch[:])
                    nc.vector.tensor_reduce(
                        lstats[:, 4 + m:5 + m], sq[:],
                        axis=mybir.AxisListType.X, op=ALU.add)

                # ship stats: [128,8] AllReduce across the 8 cores
                cb_in = dpool.tile([128, 8], F32)
                cb_out = dpool.tile([128, 8], F32)
                nc.gpsimd.dma_start(cb_in[:], lstats[:])
                nc.gpsimd.collective_compute(
                    "AllReduce",
                    ALU.add,
                    replica_groups=[list(range(N_CORES))],
                    ins=[cb_in[:].opt()],
                    outs=[cb_out[:].opt()],
                )
                gstats = stpool.tile([128, 8], F32)
                nc.gpsimd.dma_start(gstats[:], cb_out[:])
                pending = (i, gstats)

            # tail: retire the last layer
            if depth == 1:
                emit_xparts()
            pi, gstats = pending
            scale4, shift4 = emit_stats_finalize(pi, gstats)
            emit_norm(pi, scale4, shift4)

            # sigmoid(logits + bc) -> out
            nc.scalar.activation(
                out_sb[:], logits_acc[:], AF.Sigmoid, bias=bct_sb[:, :])
            nc.sync.dma_start(out_d[:], out_sb[:])

    nc.compile()
    return nc


def _get_nc(depth=DEPTH):
    if depth not in _BUILD_CACHE:
        _BUILD_CACHE[depth] = _build(depth)
    return _BUILD_CACHE[depth]


def _prep_core_inputs(c, depth, x, Ws, gamma, beta, Wc, bc, noise):
    n_chunks = 4 * (depth + 1)
    s = slice(c * B_LOC, (c + 1) * B_LOC)
    m = {}
    m["xt"] = np.ascontiguousarray(x[s].T).astype(MM_NP)
    for i in range(depth):
        m[f"wt{i}"] = np.ascontiguousarray(Ws[i].T).astype(MM_NP)
    m["noiset"] = np.ascontiguousarray(noise[:depth, s].transpose(0, 2, 1))
    wc_used = Wc[0, :128 * n_chunks]
    m["wct"] = np.ascontiguousarray(
        wc_used.reshape(n_chunks, 128).T).astype(MM_NP)
    m["gammat"] = np.ascontiguousarray(gamma[:depth].reshape(depth * 4, 128).T)
    m["betat"] = np.ascontiguousarray(beta[:depth].reshape(depth * 4, 128).T)
    m["bct"] = np.asarray(bc, dtype=np.float32).reshape(1, 1)
    return m


def _run(depth, x, Ws, gamma, beta, Wc, bc, noise):
    global LAST_EXEC_NS, LAST_RESULTS
    nc = _get_nc(depth)
    # weights/constants identical across cores: build once, reuse views
    base = _prep_core_inputs(0, depth, x, Ws, gamma, beta, Wc, bc, noise)
    in_maps = [base]
    for c in range(1, N_CORES):
        m = dict(base)
        s = slice(c * B_LOC, (c + 1) * B_LOC)
        m["xt"] = np.ascontiguousarray(x[s].T).astype(MM_NP)
        m["noiset"] = np.ascontiguousarray(
            noise[:depth, s].transpose(0, 2, 1))
        in_maps.append(m)
    kwargs = {}
    if TRACE:
        kwargs["trace"] = True
    res = bass_utils.run_bass_kernel_spmd(
        nc, in_maps, core_ids=list(range(N_CORES)), **kwargs)
    LAST_EXEC_NS = res.exec_time_ns
    LAST_RESULTS = res
    out = np.empty((B, 1), dtype=np.float32)
    for c in range(N_CORES):
        out[c * B_LOC:(c + 1) * B_LOC, 0] = res.results[c]["out"][0]
    return out


def kernel(x, W0, W1, W2, W3, W4, W5, W6, b, gamma, beta, Wc, bc, noise):
    Ws = (W0, W1, W2, W3, W4, W5, W6)
    # note: the linear bias b cancels exactly inside BatchNorm (training
    # mode) and therefore does not influence the output.
    return _run(DEPTH, np.asarray(x, np.float32),
                [np.asarray(w, np.float32) for w in Ws],
                np.asarray(gamma, np.float32), np.asarray(beta, np.float32),
                np.asarray(Wc, np.float32), np.asarray(bc, np.float32),
                np.asarray(noise, np.float32))


# -- reduced-depth entry point used by the local test harness only ----------
def kernel_depth(depth, x, Ws, gamma, beta, Wc, bc, noise):
    return _run(depth, x, list(Ws), gamma, beta, Wc, bc, noise)


# revision 23
# speedup vs baseline: 1.0447x; 1.0447x over previous
"""Trainium2 Bass kernel for nn_DiscriminatorLatent (dense MLP discriminator).

Strategy (pure data parallel over 8 NeuronCores, per the sharding hint):
 - batch dim (8192) sharded 1024 rows/core; weights replicated.
 - All on-chip tensors are feature-major (transposed): last^T is kept in SBUF
   as 128x1024 chunks; layer matmuls compute z^T = (W^T).T @ last^T in bf16
   with fp32 PSUM accumulation.
 - Each layer's matmul is split: A-phase contracts feature chunks that were
   finalized at least one layer ago (partials drained to SBUF fp32), B-phase
   contracts the most recent layer's chunks and adds the partial back.  This
   keeps the PE busy on A-phase work of the *next* layer while the current
   layer's BatchNorm AllReduce is in flight.
 - BatchNorm batch stats: per-core per-feature sum / sum-of-squares computed
   on ACT (fused into the PSUM drain + a square pass, both fp32-exact), then
   one [128,8] AllReduce per layer across the 8 cores -> exact full-batch
   statistics.  The linear bias b cancels exactly inside training-mode BN,
   so it is not applied; gamma/beta are applied.
 - LeakyReLU fused with the BN affine on ACT (Lrelu(z*scale+shift)); noise
   multiply on DVE (noise stays fp32, pre-transposed host-side so every DMA
   is contiguous).
 - Final logits: Wc contributions accumulated incrementally (M=1 matmuls per
   finished feature chunk) into PSUM, summed in SBUF, sigmoid on ACT.
"""

import os
import sys

if "/opt/trn_rl_repo" not in sys.path:
    sys.path.insert(0, "/opt/trn_rl_repo")

import ml_dtypes
import numpy as np

import concourse.bass as bass
import concourse.tile as tile
from concourse import bacc, mybir
from concourse import bass_utils

F32 = mybir.dt.float32
F32R = mybir.dt.float32r
BF16 = mybir.dt.bfloat16
AF = mybir.ActivationFunctionType
ALU = mybir.AluOpType

N_CORES = 8
B = 8192
B_LOC = B // N_CORES  # 1024
LVS = 512
WIDTH = 512
DEPTH = 7
EPS = 1e-5
SLOPE = 0.01
NH = B_LOC // 512  # moving-operand halves per z tile (N max = 512 per bank)

MM_DT = BF16  # matmul dtype for weights/activations (PSUM stays fp32)
MM_NP = ml_dtypes.bfloat16

# module-level knobs for the test harness
TRACE = False
LAST_EXEC_NS = None
LAST_RESULTS = None

_BUILD_CACHE = {}


def _build(depth=DEPTH):
    """Build + compile the SPMD bass program for `depth` layers."""
    nc = bacc.Bacc("TRN2", target_bir_lowering=False, debug=False,
                   num_devices=N_CORES)

    n_chunks = 4 * (depth + 1)  # 128-feature chunks in final last^T

    # ---- DRAM I/O ----------------------------------------------------------
    xt_d = nc.dram_tensor("xt", [LVS, B_LOC], MM_DT, kind="ExternalInput").ap()
    wt_d = [
        nc.dram_tensor(f"wt{i}", [LVS + WIDTH * i, WIDTH], MM_DT,
                       kind="ExternalInput").ap()
        for i in range(depth)
    ]
    noiset_d = nc.dram_tensor("noiset", [depth, WIDTH, B_LOC], F32,
                              kind="ExternalInput").ap()
    wct_d = nc.dram_tensor("wct", [128, n_chunks], MM_DT,
                           kind="ExternalInput").ap()
    gammat_d = nc.dram_tensor("gammat", [128, 4 * depth], F32,
                              kind="ExternalInput").ap()
    betat_d = nc.dram_tensor("betat", [128, 4 * depth], F32,
                             kind="ExternalInput").ap()
    bct_d = nc.dram_tensor("bct", [1, 1], F32, kind="ExternalInput").ap()
    out_d = nc.dram_tensor("out", [1, B_LOC], F32, kind="ExternalOutput").ap()

    # ---- persistent SBUF ---------------------------------------------------
    lastT = [
        nc.alloc_sbuf_tensor(f"lastT{k}", [128, B_LOC], MM_DT).ap()
        for k in range(n_chunks)
    ]
    wct_sb = nc.alloc_sbuf_tensor("wct_sb", [128, n_chunks], MM_DT).ap()
    gammat_sb = nc.alloc_sbuf_tensor("gammat_sb", [128, 4 * depth], F32).ap()
    betat_sb = nc.alloc_sbuf_tensor("betat_sb", [128, 4 * depth], F32).ap()
    bct_sb = nc.alloc_sbuf_tensor("bct_sb", [1, 1], F32).ap()
    logits_acc = nc.alloc_sbuf_tensor("logits_acc", [1, B_LOC], F32).ap()
    out_sb = nc.alloc_sbuf_tensor("out_sb", [1, B_LOC], F32).ap()

    with tile.TileContext(nc) as tc:
        with (
            tc.tile_pool(name="wpool", bufs=34) as wpool,
            tc.tile_pool(name="npool", bufs=4) as npool,
            tc.tile_pool(name="ppool", bufs=12) as ppool,
            tc.tile_pool(name="spool", bufs=2) as spool,
            tc.tile_pool(name="stpool", bufs=4) as stpool,
            tc.tile_pool(name="fpool", bufs=16) as fpool,
            tc.tile_pool(name="zpool", bufs=3, space="PSUM") as zpool,
            tc.tile_pool(name="lpool", bufs=1, space="PSUM") as lpool,
            tc.tile_pool(name="dpool", bufs=4, space="DRAM") as dpool,
            tc.tile_pool(name="xdpool", bufs=12, space="DRAM") as xdpool,
        ):
            # ---- preload constants + x ----
            eps_t = nc.alloc_sbuf_tensor("const_eps", [128, 1], F32)
            nc.gpsimd.memset(eps_t.ap(), EPS)
            nc.const_aps.aps[(F32, EPS)] = eps_t.ap()
            dum = nc.alloc_sbuf_tensor("dum", [128, 1], F32)
            nc.scalar.activation(dum.ap()[:], eps_t.ap()[:], AF.Sqrt, bias=EPS)
            nc.scalar.activation(dum.ap()[:], eps_t.ap()[:], AF.Lrelu,
                                 bias=0.0, scale=1.0, alpha=SLOPE)
            for k in range(4):
                nc.sync.dma_start(lastT[k][:], xt_d[k * 128:(k + 1) * 128, :])
            nc.sync.dma_start(wct_sb[:], wct_d[:])
            nc.sync.dma_start(gammat_sb[:], gammat_d[:])
            nc.sync.dma_start(betat_sb[:], betat_d[:])
            nc.sync.dma_start(bct_sb[:], bct_d[:])

            def emit_logits(group):
                """Accumulate Wc contribution of chunks 4g..4g+3 into logits."""
                lp = lpool.tile([1, B_LOC], F32)
                for jj in range(4):
                    j = 4 * group + jj
                    for h in range(NH):
                        nc.tensor.matmul(
                            lp[:, h * 512:(h + 1) * 512],
                            wct_sb[:, j:j + 1],
                            lastT[j][:, h * 512:(h + 1) * 512],
                            start=(jj == 0),
                            stop=(jj == 3),
                        )
                if group == 0:
                    nc.vector.tensor_copy(logits_acc[:], lp[:])
                else:
                    nc.vector.tensor_add(logits_acc[:], logits_acc[:], lp[:])

            def load_wblocks(i, ks):
                """Load contiguous [128,512] row-blocks k of W_i^T."""
                tiles = {}
                for k in ks:
                    wt = wpool.tile([128, WIDTH], MM_DT)
                    nc.sync.dma_start(
                        wt[:], wt_d[i][k * 128:(k + 1) * 128, :])
                    tiles[k] = wt
                return tiles

            def mm_accum_multi(psum_ts, wtiles, ms, ks):
                """Interleave accumulation of several m-tiles per k-block so
                the PE consumes each freshly-DMA'd weight block 3x slower
                than a single m-chain would (avoids burst starvation)."""
                for idx, k in enumerate(ks):
                    for m in ms:
                        for h in range(NH):
                            nc.tensor.matmul(
                                psum_ts[m][:, h * 512:(h + 1) * 512],
                                wtiles[k][:, m * 128:(m + 1) * 128],
                                lastT[k][:, h * 512:(h + 1) * 512],
                                start=(idx == 0),
                                stop=(idx == len(ks) - 1),
                            )

            def mm_accum(psum_t, wtiles, m, ks):
                for idx, k in enumerate(ks):
                    for h in range(NH):
                        nc.tensor.matmul(
                            psum_t[:, h * 512:(h + 1) * 512],
                            wtiles[k][:, m * 128:(m + 1) * 128],
                            lastT[k][:, h * 512:(h + 1) * 512],
                            start=(idx == 0),
                            stop=(idx == len(ks) - 1),
                        )

            def emit_norm(i, scale4, shift4):
                """normalize + LeakyReLU + noise + logits contribution for
                layer i's output chunks (per-chunk, so downstream consumers
                unblock as early as possible)."""
                new0 = 4 * (i + 1)
                lp = lpool.tile([1, B_LOC], F32)
                for m in range(4):
                    ch = lastT[new0 + m]
                    nc.scalar.activation(
                        ch[:], ch[:], AF.Lrelu,
                        bias=shift4[:, m:m + 1],
                        scale=scale4[:, m:m + 1],
                        alpha=SLOPE,
                    )
                    ntile = npool.tile([128, B_LOC], F32)
                    nc.scalar.dma_start(
                        ntile[:],
                        noiset_d[i:i + 1, m * 128:(m + 1) * 128, :],
                    )
                    nc.vector.tensor_mul(ch[:], ch[:], ntile[:])
                    for h in range(NH):
                        nc.tensor.matmul(
                            lp[:, h * 512:(h + 1) * 512],
                            wct_sb[:, new0 + m:new0 + m + 1],
                            ch[:, h * 512:(h + 1) * 512],
                            start=(m == 0),
                            stop=(m == 3),
                        )
                nc.vector.tensor_add(logits_acc[:], logits_acc[:], lp[:])

            def emit_stats_finalize(i, gstats):
                mean4 = fpool.tile([128, 4], F32)
                ex24 = fpool.tile([128, 4], F32)
                msq4 = fpool.tile([128, 4], F32)
                var4 = fpool.tile([128, 4], F32)
                std4 = fpool.tile([128, 4], F32)
                rstd4 = fpool.tile([128, 4], F32)
                scale4 = fpool.tile([128, 4], F32)
                nms4 = fpool.tile([128, 4], F32)
                shift4 = fpool.tile([128, 4], F32)
                nc.vector.tensor_scalar_mul(mean4[:], gstats[:, 0:4], 1.0 / B)
                nc.vector.tensor_scalar_mul(ex24[:], gstats[:, 4:8], 1.0 / B)
                nc.vector.tensor_mul(msq4[:], mean4[:], mean4[:])
                nc.vector.tensor_sub(var4[:], ex24[:], msq4[:])
                nc.scalar.activation(std4[:], var4[:], AF.Sqrt, bias=EPS)
                nc.vector.reciprocal(rstd4[:], std4[:])
                nc.vector.tensor_mul(
                    scale4[:], rstd4[:], gammat_sb[:, 4 * i:4 * i + 4])
                nc.vector.scalar_tensor_tensor(
                    nms4[:], mean4[:], -1.0, scale4[:],
                    op0=ALU.mult, op1=ALU.mult)
                nc.vector.tensor_add(
                    shift4[:], nms4[:], betat_sb[:, 4 * i:4 * i + 4])
                return scale4, shift4

            xparts = {}
            xparts_dram = {}

            def emit_xparts():
                """x's logits + pre-contraction of the x chunks of all later
                layers.  Emitted after layer 0 ships its stats: fills the PE
                during the collectives init barrier + first AllReduce.
                Layers 2/3 keep their partials in SBUF; 4+ bounce via DRAM."""
                emit_logits(0)
                for j in range(3, depth):
                    wtiles_x = load_wblocks(j, range(4))
                    tgt = xparts if j < 4 else xparts_dram
                    tgt[j] = {}
                    for m in range(4):
                        xt_ps = zpool.tile([128, B_LOC], F32, tag="z")
                        mm_accum(xt_ps, wtiles_x, m, range(4))
                        xp = ppool.tile([128, B_LOC], F32, tag="pt")
                        nc.vector.tensor_copy(xp[:], xt_ps[:])
                        if j < 4:
                            tgt[j][m] = xp
                        else:
                            xd = xdpool.tile([128, B_LOC], F32)
                            nc.gpsimd.dma_start(xd[:], xp[:])
                            tgt[j][m] = xd

            # ---- layer pipeline ----
            # A-phase of layer i: chunks 0..4i-1 (ready >= one layer ago)
            # B-phase of layer i: chunks 4i..4i+3 (previous layer's output)
            pending = None  # (i, lstats) shipped to AllReduce, not yet normed
            for i in range(depth):
                if i == 1:
                    emit_xparts()
                xpart = xparts.get(i)
                xpart_d = xparts_dram.get(i)
                old_ks = list(range(4 if (xpart or xpart_d) else 0, 4 * i))
                new_ks = list(range(4 * i, 4 * (i + 1)))

                # A-phase (independent of the pending AllReduce).  The m=3
                # drain is deferred past the retire chain so the DVE queue
                # head is free the moment the AllReduce lands.
                deferred_drain = None
                if old_ks:
                    wtiles_a = load_wblocks(i, old_ks)
                    if xpart_d:
                        xpart = {}
                        for m in range(4):
                            pt = ppool.tile([128, B_LOC], F32, tag="pt")
                            nc.gpsimd.dma_start(pt[:], xpart_d[m][:])
                            xpart[m] = pt
                    ats = {}
                    for m in range(3):
                        at_t = zpool.tile([128, B_LOC], F32, tag="z")
                        ats[m] = at_t
                    mm_accum_multi(ats, wtiles_a, (0, 1, 2), old_ks)
                    new_partials = {}
                    for m in range(3):
                        if xpart:
                            pt = xpart[m]
                            nc.vector.tensor_add(pt[:], pt[:], ats[m][:])
                        else:
                            pt = ppool.tile([128, B_LOC], F32, tag="pt")
                            nc.vector.tensor_copy(pt[:], ats[m][:])
                        new_partials[m] = pt
                    at3 = zpool.tile([128, B_LOC], F32, tag="z")
                    mm_accum(at3, wtiles_a, 3, old_ks)
                    if xpart:
                        deferred_drain = ("add", at3, xpart[3])
                        new_partials[3] = xpart[3]
                    else:
                        pt3 = ppool.tile([128, B_LOC], F32, tag="pt")
                        deferred_drain = ("copy", at3, pt3)
                        new_partials[3] = pt3
                else:
                    new_partials = xpart

                # retire the pending AllReduce: finalize + normalize + logits
                if pending is not None:
                    pi, gstats = pending
                    scale4, shift4 = emit_stats_finalize(pi, gstats)
                    emit_norm(pi, scale4, shift4)
                    pending = None
                if deferred_drain is not None:
                    kind, at, pt = deferred_drain
                    if kind == "add":
                        nc.vector.tensor_add(pt[:], pt[:], at[:])
                    else:
                        nc.vector.tensor_copy(pt[:], at[:])

                # B-phase: newest chunks + partial add, then stats
                wtiles_b = load_wblocks(i, new_ks)
                lstats = stpool.tile([128, 8], F32)
                for m in range(4):
                    bt = zpool.tile([128, B_LOC], F32, tag="z")
                    mm_accum(bt, wtiles_b, m, new_ks)
                    ch = lastT[4 * (i + 1) + m]
                    if new_partials is not None:
                        nc.vector.tensor_tensor(
                            ch[:], bt[:], new_partials[m][:], op=ALU.add)
                    else:
                        nc.vector.tensor_copy(ch[:], bt[:])
                    nc.vector.tensor_reduce(
                        lstats[:, m:m + 1], ch[:],
                        axis=mybir.AxisListType.X, op=ALU.add)
                    sq = spool.tile([128, B_LOC], BF16)
                    nc.gpsimd.tensor_mul(sq[:], ch[:], ch[:])
                    nc.vector.tensor_reduce(
                        lstats[:, 4 + m:5 + m], sq[:],
                        axis=mybir.AxisListType.X, op=ALU.add)

                # ship stats: [128,8] AllReduce across the 8 cores
                cb_in = dpool.tile([128, 8], F32)
                cb_out = dpool.tile([128, 8], F32)
                nc.gpsimd.dma_start(cb_in[:], lstats[:])
                nc.gpsimd.collective_compute(
                    "AllReduce",
                    ALU.add,
                    replica_groups=[list(range(N_CORES))],
                    ins=[cb_in[:].opt()],
                    outs=[cb_out[:].opt()],
                )
                gstats = stpool.tile([128, 8], F32)
                nc.gpsimd.dma_start(gstats[:], cb_out[:])
                pending = (i, gstats)

            # tail: retire the last layer
            if depth == 1:
                emit_xparts()
            pi, gstats = pending
            scale4, shift4 = emit_stats_finalize(pi, gstats)
            emit_norm(pi, scale4, shift4)

            # sigmoid(logits + bc) -> out
            nc.scalar.activation(
                out_sb[:], logits_acc[:], AF.Sigmoid, bias=bct_sb[:, :])
            nc.sync.dma_start(out_d[:], out_sb[:])

    nc.compile()
    return nc


def _get_nc(depth=DEPTH):
    if depth not in _BUILD_CACHE:
        _BUILD_CACHE[depth] = _build(depth)
    return _BUILD_CACHE[depth]


def _prep_core_inputs(c, depth, x, Ws, gamma, beta, Wc, bc, noise):
    n_chunks = 4 * (depth + 1)
    s = slice(c * B_LOC, (c + 1) * B_LOC)
    m = {}
    m["xt"] = np.ascontiguousarray(x[s].T).astype(MM_NP)
    for i in range(depth):
        m[f"wt{i}"] = np.ascontiguousarray(Ws[i].T).astype(MM_NP)
    m["noiset"] = np.ascontiguousarray(noise[:depth, s].transpose(0, 2, 1))
    wc_used = Wc[0, :128 * n_chunks]
    m["wct"] = np.ascontiguousarray(
        wc_used.reshape(n_chunks, 128).T).astype(MM_NP)
    m["gammat"] = np.ascontiguousarray(gamma[:depth].reshape(depth * 4, 128).T)
    m["betat"] = np.ascontiguousarray(beta[:depth].reshape(depth * 4, 128).T)
    m["bct"] = np.asarray(bc, dtype=np.float32).reshape(1, 1)
    return m


def _run(depth, x, Ws, gamma, beta, Wc, bc, noise):
    global LAST_EXEC_NS, LAST_RESULTS
    nc = _get_nc(depth)
    # weights/constants identical across cores: build once, reuse views
    base = _prep_core_inputs(0, depth, x, Ws, gamma, beta, Wc, bc, noise)
    in_maps = [base]
    for c in range(1, N_CORES):
        m = dict(base)
        s = slice(c * B_LOC, (c + 1) * B_LOC)
        m["xt"] = np.ascontiguousarray(x[s].T).astype(MM_NP)
        m["noiset"] = np.ascontiguousarray(
            noise[:depth, s].transpose(0, 2, 1))
        in_maps.append(m)
    kwargs = {}
    if TRACE:
        kwargs["trace"] = True
    res = bass_utils.run_bass_kernel_spmd(
        nc, in_maps, core_ids=list(range(N_CORES)), **kwargs)
    LAST_EXEC_NS = res.exec_time_ns
    LAST_RESULTS = res
    out = np.empty((B, 1), dtype=np.float32)
    for c in range(N_CORES):
        out[c * B_LOC:(c + 1) * B_LOC, 0] = res.results[c]["out"][0]
    return out


def kernel(x, W0, W1, W2, W3, W4, W5, W6, b, gamma, beta, Wc, bc, noise):
    Ws = (W0, W1, W2, W3, W4, W5, W6)
    # note: the linear bias b cancels exactly inside BatchNorm (training
    # mode) and therefore does not influence the output.
    return _run(DEPTH, np.asarray(x, np.float32),
                [np.asarray(w, np.float32) for w in Ws],
                np.asarray(gamma, np.float32), np.asarray(beta, np.float32),
                np.asarray(Wc, np.float32), np.asarray(bc, np.float32),
                np.asarray(noise, np.float32))


# -- reduced-depth entry point used by the local test harness only ----------
def kernel_depth(depth, x, Ws, gamma, beta, Wc, bc, noise):
    return _run(depth, x, list(Ws), gamma, beta, Wc, bc, noise)
